# revision 37
# baseline (speedup 1.0000x reference)
"""Trainium2 Bass kernel for 6-head causal self-attention (nn_MultiHeadAttention).

Full-input contract: kernel(**inputs) takes the unsharded numpy inputs and
returns the full [16, 2048, 384] output. Internally the batch dim (16) is
sharded 2-per-core across 8 NeuronCores (data parallel, no collectives).

The kernel is ScalarE(exp)-bound: causal softmax needs ~27M exp elements per
core at 1 elem/lane/cycle. Structure is a single flat software pipeline over
all (batch, head-pair, query-block, key-tile) steps:

  step k:  S(k+1) on PE  |  exp(k) on ACT  |  U(k-1) on PE

so the ACT exp stream never waits at query-block / pair / batch boundaries.
S^T tiles are [s=128, 2 heads, 512 q] in PSUM (2 banks, double-buffered);
K=64 head-pair S-matmuls run concurrently via row-group tiling. U^T = V^T@P^T
accumulates per query-block in PSUM ([V_h | ones] stationary also yields
softmax row-sums for free); diagonal key-tiles use narrowed column windows.
Row-sums are packed two-heads-per-partition-half and normalized with a single
ln + exp(-x) pair over [128, 2048] per head-pair (both live in one ACT table
set). QKV / output projections are deferred PE work drained by a deficit
pacer that keeps TensorE from idling (HAM stays warm) without ever delaying
the exp stream; a gradual 1-per-step prefetch retires the next block's qk
units before they are needed. Output is written bf16 (tolerance is 2e-2).

Measured: 402us baseline -> 317us (ScalarE ~78% busy, HAM throttle ~20us).
"""

import sys

for _p in ("/opt/trn_rl_repo",):
    if _p not in sys.path:
        sys.path.insert(0, _p)

import numpy as np

B, T, C = 16, 2048, 384
H, DH = 6, 64
NCORES = 8
BPC = B // NCORES  # batches per core
KC = C // 128      # 3 contraction chunks
NTQ = T // 512     # 4 query blocks
NSI = T // 128     # 16 key tiles

_CACHE = {}

# pacer tuning (ns)
ACT_EXP_OVH = 300.0
PE_UNIT = 700.0
PACE_SLACK = 50.0
PE_S_OVH = 40.0
PE_U_OVH = 60.0


def _build():
    if "nc" in _CACHE:
        return _CACHE["nc"]

    import bass_rust as _bass_rust
    import concourse.bacc as bacc
    import concourse.mybir as mybir
    import concourse.tile as tile
    from concourse.hw_specs import get_activation_tables

    dt = mybir.dt
    AF = mybir.ActivationFunctionType
    OP = mybir.AluOpType

    class _Bacc(bacc.Bacc):
        # This kernel only uses Exp and Ln on ScalarE. Both live in the
        # natural_log_exp_and_others table set; without this filter the
        # table picker alternates between exp-only and ln+exp sets,
        # inserting an ACT_TABLE_LOAD (~1.5us) per switch.
        def insert_act_table_loads(self):
            has_activation = any(
                isinstance(i, mybir.InstActivation)
                for b in self.main_func.blocks
                for i in b.instructions
            )
            if not has_activation:
                return
            keep = {"natural_log_exp_and_others"}
            tables = [
                (n, (s if n in keep else (s - {AF.Exp, AF.Ln})))
                for n, s in get_activation_tables(self.m.arch).items()
            ]
            _bass_rust.insert_act_table_loads(self, tables)

    nc = _Bacc("TRN2", target_bir_lowering=False, debug=True)

    xT_d = nc.dram_tensor("xT", [BPC, KC, 128, T], dt.bfloat16, kind="ExternalInput")
    wqk_d = nc.dram_tensor("Wqk", [KC, 128, 768], dt.bfloat16, kind="ExternalInput")
    wv_d = nc.dram_tensor("Wv", [KC, 128, 384], dt.bfloat16, kind="ExternalInput")
    wo_d = nc.dram_tensor("Wo", [KC, 128, 384], dt.bfloat16, kind="ExternalInput")
    bo_d = nc.dram_tensor("bo", [KC, 128, 1], dt.float32, kind="ExternalInput")
    yT_d = nc.dram_tensor("yT", [BPC, KC, 128, T], dt.bfloat16, kind="ExternalOutput")

    with tile.TileContext(nc) as tc:
        with (
            tc.tile_pool(name="wp", bufs=1) as wp,
            tc.tile_pool(name="vp", bufs=2) as vp,
            tc.tile_pool(name="xp", bufs=2) as xp,
            tc.tile_pool(name="pp", bufs=6) as pp,
            tc.tile_pool(name="np_", bufs=2) as np_,
            tc.tile_pool(name="yp", bufs=2) as yp,
            tc.tile_pool(name="ups", bufs=3, space="PSUM") as ups,
            tc.tile_pool(name="mm", bufs=1, space="PSUM") as mm,
            tc.tile_pool(name="sp", bufs=2, space="PSUM") as sp,
        ):
            # ---- constants ----
            # issue order matters: the sync queue serializes dma_start at
            # ~0.6us each, and exp(0) waits on wqk+xt(0) via the first qk
            # units. wv next (first U needs v0); wo/bo are needed late.
            wqk = wp.tile([128, KC, 768], dt.bfloat16, name="wqk")
            wv = wp.tile([128, KC, 384], dt.bfloat16, name="wv")
            wo = wp.tile([128, KC, 384], dt.bfloat16, name="wo")
            bo = wp.tile([128, KC], dt.float32, name="bo")

            def load_weights_early():
                for k in range(KC):
                    nc.sync.dma_start(wqk[:, k], wqk_d[k])

            def load_weights_late():
                for k in range(KC):
                    nc.sync.dma_start(wv[:, k], wv_d[k])
                for k in range(KC):
                    nc.sync.dma_start(wo[:, k], wo_d[k])
                for k in range(KC):
                    nc.sync.dma_start(bo[:, k, None], bo_d[k])

            # ---- flat step schedule ----
            steps = []
            pair_start = {}
            pair_list = []
            for b in range(BPC):
                for p in range(3):
                    pair_start[(b, p)] = len(steps)
                    pair_list.append((b, p))
                    for qb in range(NTQ):
                        nsi = 4 * qb + 4
                        for si in range(nsi):
                            steps.append((b, p, qb, si, nsi))
            NS = len(steps)

            # ---- pipeline state ----
            state = {"act": 0.0, "pe": 0.0}
            fillers = []
            qk_done = {}   # (b, p) -> completed qk units
            v_done = {}    # b -> completed v units
            bctx = {}      # b -> dict(xt, vones, qt, kt, ot)
            sps_map = {}
            pt_map = {}
            qbu = {}       # (b, p, qb) -> (u0, u1)
            pairctx = {}   # (b, p) -> (uw, sm, ot)
            hooks = {}     # k -> [closures] run after emit_exp(k)
            started = set()

            def add_hook(k, fn):
                hooks.setdefault(k, []).append(fn)

            def drain(n=1):
                for _ in range(n):
                    if fillers:
                        fillers.pop(0)()
                        state["pe"] += PE_UNIT

            def pace(allowed=True, loose=False):
                # engines cannot bank more than a queue's worth of surplus:
                # clamp stale credit so the pacer reacts to recent history
                state["pe"] = min(state["pe"], state["act"] + 1500.0)
                state["act"] = min(state["act"], state["pe"] + 4000.0)
                if loose:
                    n = 0
                    while fillers and n < 2 and state["pe"] <= state["act"] + 1000.0:
                        fillers.pop(0)()
                        state["pe"] += PE_UNIT
                        n += 1
                    return
                if not allowed:
                    return
                while fillers and state["pe"] + PE_UNIT <= state["act"] - PACE_SLACK:
                    fillers.pop(0)()
                    state["pe"] += PE_UNIT

            # ---- deferred PE units ----
            def v_unit(b, ti):
                def emit():
                    ctx = bctx[b]
                    vones = ctx["vones"]
                    nc.gpsimd.memset(vones[:, ti, :, 64:128], 1.0)
                    ps = mm.tile([128, 512], dt.float32, name="ps_mm")
                    for k in range(KC):
                        nc.tensor.matmul(
                            ps[:, 0:384],
                            ctx["xt"][:, k, 128 * ti : 128 * ti + 128],
                            wv[:, k, :],
                            start=(k == 0),
                            stop=(k == KC - 1),
                        )
                    nc.vector.tensor_copy(
                        out=vones[:, ti, :, 0:64], in_=ps[:, 0:384]
                    )
                    v_done[b] = v_done.get(b, 0) + 1
                return emit

            def qk_unit(b, p, tq, qk):
                def emit():
                    ctx = bctx[b]
                    ps = mm.tile([128, 512], dt.float32, name="ps_mm")
                    for k in range(KC):
                        nc.tensor.matmul(
                            ps[:],
                            wqk[:, k, 256 * p + 128 * qk : 256 * p + 128 * qk + 128],
                            ctx["xt"][:, k, 512 * tq : 512 * tq + 512],
                            start=(k == 0),
                            stop=(k == KC - 1),
                        )
                    dst = ctx["qt"] if qk == 0 else ctx["kt"]
                    nc.vector.tensor_copy(
                        out=dst[:, p, 512 * tq : 512 * tq + 512], in_=ps[:]
                    )
                    qk_done[(b, p)] = qk_done.get((b, p), 0) + 1
                return emit

            tail_state = {"on": False, "i": 0}

            def oproj_unit(b, ot, tq, mo, pool=None, pname=None):
                def emit():
                    pl, pn = pool, pname
                    if pl is None and tail_state["on"]:
                        # after attention ends the ups banks are free: cycle
                        # ups(3)+mm(1) so the tail projections pipeline
                        pl, pn = ([(ups, "ps_u")] * 3 + [(mm, "ps_mm")])[
                            tail_state["i"] % 4
                        ]
                        tail_state["i"] += 1
                    ps = (pl if pl is not None else mm).tile(
                        [128, 512], dt.float32, name=(pn or "ps_mm")
                    )
                    for k in range(KC):
                        nc.tensor.matmul(
                            ps[:],
                            wo[:, k, 128 * mo : 128 * mo + 128],
                            ot[:, k, 512 * tq : 512 * tq + 512],
                            start=(k == 0),
                            stop=(k == KC - 1),
                        )
                    yt = yp.tile([128, 512], dt.bfloat16, name="yt")
                    nc.vector.tensor_tensor(
                        out=yt[:],
                        in0=ps[:],
                        in1=bo[:, mo, None].to_broadcast([128, 512]),
                        op=OP.add,
                    )
                    nc.sync.dma_start(
                        yT_d[b, mo, :, 512 * tq : 512 * tq + 512], yt[:]
                    )
                return emit

            def make_bctx(b):
                xt = xp.tile([128, KC, T], dt.bfloat16, name="xt")
                for k in range(KC):
                    nc.sync.dma_start(xt[:, k], xT_d[b, k])
                vones = vp.tile([128, NSI, H, 128], dt.bfloat16, name="vones")
                qt = xp.tile([128, 3, T], dt.bfloat16, name="qt")
                kt = xp.tile([128, 3, T], dt.bfloat16, name="kt")
                ot = xp.tile([128, 3, T], dt.bfloat16, name="ot")
                bctx[b] = dict(xt=xt, vones=vones, qt=qt, kt=kt, ot=ot)

            # ---- pipeline stages ----
            def emit_S(j):
                b, p, qb, si, nsi = steps[j]
                ensure_pair(b, p)
                need = 2 * (qb + 1)
                while qk_done.get((b, p), 0) < need:
                    assert fillers, "filler underrun for qk units"
                    fillers.pop(0)()
                    state["pe"] += PE_UNIT
                ctx = bctx[b]
                diag = si >= 4 * qb
                lo = 128 * (si - 4 * qb) if diag else 0
                sps = sp.tile([128, 2, 512], dt.float32, name="sps")
                for hf in range(2):
                    nc.tensor.matmul(
                        sps[:, hf, lo:512],
                        ctx["kt"][64 * hf : 64 * hf + 64, p,
                                  128 * si : 128 * si + 128],
                        ctx["qt"][64 * hf : 64 * hf + 64, p,
                                  512 * qb + lo : 512 * qb + 512],
                        start=True,
                        stop=True,
                    )
                sps_map[j] = (sps, lo)
                state["pe"] += (512 - lo) / 2.4 + PE_S_OVH
                if (qb, si) in POLY and si >= 4 * qb:
                    raise AssertionError("POLY must be non-diagonal")
                if (qb, si) in POLY:
                    # quadratic-softmax offload, emitted a full step early so
                    # the DVE chain (which also frees this sps buffer) has
                    # pipeline slack; exp(j) then becomes a no-op on ACT
                    ptt = pp.tile([128, 2, 512], dt.bfloat16, name="ptt")
                    t = pp.tile([128, 2, 512], dt.bfloat16, name="pscr", bufs=3)
                    sq = pp.tile([128, 2, 512], dt.bfloat16, name="pscr", bufs=3)
                    nc.vector.tensor_scalar_add(t[:], sps[:], 8.0)
                    nc.vector.tensor_tensor(
                        out=sq[:], in0=t[:], in1=t[:], op=OP.mult
                    )
                    nc.gpsimd.tensor_scalar(
                        ptt[:], sq[:], 0.0078125, 0.5, OP.mult, OP.add
                    )
                    pt_map[j] = (ptt, 0)

            # (qb, si) steps whose exp is offloaded to VectorE as the exact
            # quadratic Taylor in the raw-score domain:
            #   exp(p/8) ~= 0.0078125*(p+8)^2 + 0.5   (scores are tiny:
            # std 0.154, |s|max ~0.86 -> rel err ~0.1% rms / 4e-3 max on U)
            POLY = set()  # poly offload loses to scheduling friction; keep exp on ACT

            def emit_exp(j):
                b, p, qb, si, nsi = steps[j]
                sps, lo = sps_map.pop(j)
                if j in pt_map:  # poly step: chain already emitted at emit_S
                    return
                ptt = pp.tile([128, 2, 512], dt.bfloat16, name="ptt")
                nc.scalar.activation(
                    ptt[:, :, lo:], sps[:, :, lo:], AF.Exp, scale=0.125
                )
                if si >= 4 * qb:
                    # zero the still-masked triangle in the 128-col
                    # diagonal window: keep iff f >= p
                    nc.gpsimd.affine_select(
                        out=ptt[:, :, lo : lo + 128],
                        in_=ptt[:, :, lo : lo + 128],
                        compare_op=OP.is_ge,
                        fill=0.0,
                        base=0,
                        channel_multiplier=-1,
                        pattern=[[0, 2], [1, 128]],
                    )
                pt_map[j] = (ptt, lo)
                state["act"] += (2 * (512 - lo) + ACT_EXP_OVH) / 1.2

            def emit_U(j):
                b, p, qb, si, nsi = steps[j]
                if v_done.get(b, 0) < NSI:
                    while v_done.get(b, 0) < si + 1:
                        assert fillers, "filler underrun for v units"
                        fillers.pop(0)()
                        state["pe"] += PE_UNIT
                ptt, lo = pt_map.pop(j)
                ctx = bctx[b]
                if si == 0:
                    u0 = ups.tile([128, 512], dt.float32, name="ps_u")
                    u1 = ups.tile([128, 512], dt.float32, name="ps_u")
                    qbu[(b, p, qb)] = (u0, u1)
                u0, u1 = qbu[(b, p, qb)]
                for hf, uu in ((0, u0), (1, u1)):
                    nc.tensor.matmul(
                        uu[:, lo:512],
                        ctx["vones"][:, si, 2 * p + hf, :],
                        ptt[:, hf, lo:512],
                        start=(si == 0),
                        stop=(si == nsi - 1),
                    )
                state["pe"] += 2 * (512 - lo) / 2.4 + PE_U_OVH
                if si == nsi - 1:
                    emit_evac(b, p, qb)

            def emit_evac(b, p, qb):
                # pack U halves and softmax row-sums two-heads-per-tile;
                # u0 copies first so its ups slot frees earliest
                if qb == 0:
                    uw = np_.tile([128, NTQ, 512], dt.bfloat16, name="uw")
                    sm = np_.tile([128, NTQ, 512], dt.bfloat16, name="sm")
                    pairctx[(b, p)] = (uw, sm, bctx[b]["ot"])
                uw, sm, _ot = pairctx[(b, p)]
                u0, u1 = qbu.pop((b, p, qb))
                nc.vector.tensor_copy(out=sm[0:64, qb], in_=u0[64:128])
                nc.vector.tensor_copy(out=uw[0:64, qb], in_=u0[0:64])
                nc.vector.tensor_copy(out=sm[64:128, qb], in_=u1[64:128])
                nc.vector.tensor_copy(out=uw[64:128, qb], in_=u1[0:64])

            def norm_pair(b, p):
                def run():
                    uw, sm, ot = pairctx.pop((b, p))
                    lnr = np_.tile([128, NTQ, 512], dt.float32, name="lnr", bufs=1)
                    rec = np_.tile([128, NTQ, 512], dt.bfloat16, name="rec", bufs=1)
                    nc.scalar.activation(lnr[:], sm[:], AF.Ln)
                    nc.scalar.activation(rec[:], lnr[:], AF.Exp, scale=-1.0)
                    state["act"] += 2 * (NTQ * 512 + ACT_EXP_OVH) / 1.2
                    for qb in range(NTQ):
                        nc.vector.tensor_tensor(
                            out=ot[:, p, 512 * qb : 512 * qb + 512],
                            in0=uw[:, qb],
                            in1=rec[:, qb],
                            op=OP.mult,
                        )
                    if p == 2:
                        for tq in range(NTQ):
                            for mo in range(KC):
                                fillers.append(oproj_unit(b, ot, tq, mo))
                return run

            def qb_norm(b, p, qb):
                # last-batch final pair: per-qb norm so oproj can drain early
                def run():
                    uw, sm, ot = pairctx[(b, p)]
                    lnq = np_.tile([128, 512], dt.float32, name="lnq")
                    recq = np_.tile([128, 512], dt.bfloat16, name="recq")
                    nc.scalar.activation(lnq[:], sm[:, qb], AF.Ln)
                    nc.scalar.activation(recq[:], lnq[:], AF.Exp, scale=-1.0)
                    state["act"] += 2 * (512 + ACT_EXP_OVH) / 1.2
                    nc.vector.tensor_tensor(
                        out=ot[:, p, 512 * qb : 512 * qb + 512],
                        in0=uw[:, qb],
                        in1=recq[:],
                        op=OP.mult,
                    )
                    if qb < NTQ - 1:
                        for mo in range(KC):
                            fillers.append(oproj_unit(b, ot, qb, mo))
                return run

            def ensure_pair(b, p):
                if (b, p) in started:
                    return
                started.add((b, p))
                if p < 2:
                    for tq in range(NTQ):
                        for qk in range(2):
                            fillers.append(qk_unit(b, p + 1, tq, qk))
                if p == 1 and b + 1 < BPC:
                    # next batch's projections, demand-first interleave
                    make_bctx(b + 1)
                    nb = b + 1
                    fillers.extend([qk_unit(nb, 0, 0, 0), qk_unit(nb, 0, 0, 1)])
                    fillers.extend([v_unit(nb, 0), v_unit(nb, 1)])
                    fillers.extend([qk_unit(nb, 0, 1, 0), qk_unit(nb, 0, 1, 1)])
                    fillers.extend([v_unit(nb, 2), v_unit(nb, 3)])
                    fillers.extend([qk_unit(nb, 0, 2, 0), qk_unit(nb, 0, 2, 1)])
                    fillers.extend([v_unit(nb, ti) for ti in range(4, 8)])
                    fillers.extend([qk_unit(nb, 0, 3, 0), qk_unit(nb, 0, 3, 1)])
                    fillers.extend([v_unit(nb, ti) for ti in range(8, NSI)])
                # schedule this pair's packed norm 2 steps into the next pair
                pi = pair_list.index((b, p))
                last_batch_final = (b == BPC - 1 and p == 2)
                if not last_batch_final:
                    if pi + 1 < len(pair_list):
                        add_hook(pair_start[pair_list[pi + 1]] + 2, norm_pair(b, p))
                else:
                    # per-qb norms, due 2 steps after each qb's evac
                    k0 = pair_start[(b, p)]
                    off = 0
                    for qb in range(NTQ):
                        nsi = 4 * qb + 4
                        off += nsi
                        due = min(k0 + off + 2, NS - 1)
                        if qb < NTQ - 1:
                            add_hook(due, qb_norm(b, p, qb))

            # ---- prologue ----
            make_bctx(0)
            load_weights_early()
            load_weights_late()
            # warm up the PE (HAM K=8/8) on scratch data during the DMA
            # wait; sized to end just before the weights land so the gap
            # stays under the ~3.4us HAM re-throttle window
            scr = wp.tile([128, 640], dt.bfloat16, name="scr")
            nc.gpsimd.memset(scr[:], 1.0)
            psw = mm.tile([128, 512], dt.float32, name="ps_mm")
            for _ in range(24):
                nc.tensor.matmul(
                    psw[:], scr[:, 0:128], scr[:, 128:640], start=True, stop=True
                )
            # first two qk units go through the (still idle) ups pool so
            # they pipeline instead of serializing on the 1-buf mm pool
            for qk in range(2):
                ps = ups.tile([128, 512], dt.float32, name="ps_u")
                for k in range(KC):
                    nc.tensor.matmul(
                        ps[:],
                        wqk[:, k, 128 * qk : 128 * qk + 128],
                        bctx[0]["xt"][:, k, 0:512],
                        start=(k == 0),
                        stop=(k == KC - 1),
                    )
                dst = bctx[0]["qt"] if qk == 0 else bctx[0]["kt"]
                nc.vector.tensor_copy(out=dst[:, 0, 0:512], in_=ps[:])
            qk_done[(0, 0)] = 2
            seed = []
            seed += [v_unit(0, 0), v_unit(0, 1)]
            seed += [qk_unit(0, 0, 1, 0), qk_unit(0, 0, 1, 1)]
            seed += [v_unit(0, 2), v_unit(0, 3)]
            seed += [qk_unit(0, 0, 2, 0), qk_unit(0, 0, 2, 1)]
            seed += [v_unit(0, ti) for ti in range(4, 8)]
            seed += [qk_unit(0, 0, 3, 0), qk_unit(0, 0, 3, 1)]
            seed += [v_unit(0, ti) for ti in range(8, NSI)]
            fillers.extend(seed)

            # ---- main pipeline ----
            emit_S(0)
            for k in range(NS):
                b, p, qb, si, nsi = steps[k]
                if k + 1 < NS:
                    emit_S(k + 1)
                emit_exp(k)
                for h in hooks.pop(k, ()):
                    h()
                # U runs TWO steps behind: the diagonal-tile U waits on its
                # gpsimd affine_select, and with a 1-step lag that wait sits
                # at the head of the in-order PE queue blocking the next
                # S-pair (the qb-boundary exp gaps); 2 steps decouples it
                if k >= 2:
                    emit_U(k - 2)
                # gradual demand prefetch: one filler per step until the
                # UPCOMING qk need (next qb, or next pair's first block) is
                # met, so boundary steps never hit a just-in-time qk burst
                if qb + 1 < NTQ:
                    need, cnt = (b, p), 2 * (qb + 2)
                else:
                    pi = pair_list.index((b, p))
                    need, cnt = (
                        (pair_list[pi + 1], 2) if pi + 1 < len(pair_list)
                        else (None, 0)
                    )
                if (
                    need is not None
                    and qk_done.get(need, 0) < cnt
                    and fillers
                    and state["pe"] < state["act"]
                ):
                    fillers.pop(0)()
                    state["pe"] += PE_UNIT
                # keep fillers flowing everywhere: banning them in diag
                # regions idles the PE long enough to re-trigger HAM
                # throttling, which costs more than the exp-stream hiccups
                pace(loose=(b == BPC - 1 and p == 2))
            emit_U(NS - 2)
            emit_U(NS - 1)

            # ---- tail: last qb norm + remaining output projections ----
            bl, pl_ = BPC - 1, 2
            qb_last = NTQ - 1
            uw, sm, ot = pairctx[(bl, pl_)]
            lnq = np_.tile([128, 512], dt.float32, name="lnq")
            recq = np_.tile([128, 512], dt.bfloat16, name="recq")
            nc.scalar.activation(lnq[:], sm[:, qb_last], AF.Ln)
            nc.scalar.activation(recq[:], lnq[:], AF.Exp, scale=-1.0)
            nc.vector.tensor_tensor(
                out=ot[:, pl_, 512 * qb_last : 512 * qb_last + 512],
                in0=uw[:, qb_last],
                in1=recq[:],
                op=OP.mult,
            )
            del pairctx[(bl, pl_)]
            # drain whatever fillers remain, then the last qb's oproj; psum
            # slots cycle over ups(3) + mm(1) so the tail pipelines
            tail_state["on"] = True
            while fillers:
                drain(1)
            for mo in range(KC):
                oproj_unit(bl, ot, qb_last, mo)()

    nc.compile()
    _CACHE["nc"] = nc
    return nc


def _prep_inputs(x, Wq, Wk, Wv, Wo, bo):
    import ml_dtypes
    bf16 = ml_dtypes.bfloat16
    x = np.ascontiguousarray(np.asarray(x, dtype=np.float32))
    Wq = np.asarray(Wq, dtype=np.float32)
    Wk = np.asarray(Wk, dtype=np.float32)
    Wv = np.asarray(Wv, dtype=np.float32)
    Wo = np.asarray(Wo, dtype=np.float32)
    bo = np.asarray(bo, dtype=np.float32)

    # x^T: [B, T, C] -> [B, C, T] -> [B, KC, 128, T]
    xT = np.ascontiguousarray(x.transpose(0, 2, 1)).reshape(B, KC, 128, T).astype(bf16)

    # Wqk columns per pair p: [Q_2p | Q_2p+1 | K_2p | K_2p+1], 64 each
    wqk = np.empty((C, 768), np.float32)
    for p in range(3):
        wqk[:, 256 * p + 0 : 256 * p + 64] = Wq[2 * p]
        wqk[:, 256 * p + 64 : 256 * p + 128] = Wq[2 * p + 1]
        wqk[:, 256 * p + 128 : 256 * p + 192] = Wk[2 * p]
        wqk[:, 256 * p + 192 : 256 * p + 256] = Wk[2 * p + 1]
    wqk = np.ascontiguousarray(wqk.reshape(KC, 128, 768)).astype(bf16)

    # Wv columns (h*64+e), rows C -> [KC, 128, 384]
    wv = np.ascontiguousarray(
        Wv.transpose(1, 0, 2).reshape(C, H * DH).reshape(KC, 128, H * DH)
    ).astype(bf16)
    wo = np.ascontiguousarray(Wo.reshape(KC, 128, C)).astype(bf16)
    bo_r = np.ascontiguousarray(bo.reshape(KC, 128, 1))
    return xT, wqk, wv, wo, bo_r


def _run(inputs, trace=False):
    from concourse.bass_utils import run_bass_kernel_spmd

    nc = _build()
    xT, wqk, wv, wo, bo_r = _prep_inputs(**inputs)
    in_maps = [
        {
            "xT": xT[BPC * i : BPC * (i + 1)],
            "Wqk": wqk,
            "Wv": wv,
            "Wo": wo,
            "bo": bo_r,
        }
        for i in range(NCORES)
    ]
    res = run_bass_kernel_spmd(nc, in_maps, list(range(NCORES)), trace=trace)
    # yT per core: [BPC, KC, 128, T] -> full y [B, T, C]
    yT = np.concatenate([np.asarray(res.results[i]["yT"]) for i in range(NCORES)], axis=0)
    y = yT.reshape(B, C, T).transpose(0, 2, 1)
    return np.ascontiguousarray(y.astype(np.float32)), res.exec_time_ns


def kernel(**inputs) -> np.ndarray:
    y, _ = _run(inputs, trace=False)
    return y
